# revision 17
# baseline (speedup 1.0000x reference)
"""Ragged packed-sequence RoPE on 8 TRN2 NeuronCores.

Strategy:
  - Token-shard: core i gets tokens [i*T/8, (i+1)*T/8). The op is
    embarrassingly parallel over tokens; no collectives.
  - Raggedness is resolved on the host: per-token rotary rows are
    sin_cache[off]/cos_cache[off] where off restarts at 0 per sequence
    (exactly replicating the reference's searchsorted/gather semantics,
    including JAX's negative-wrap + clip out-of-bounds behavior). The
    gathered [T, 2*R] sin|cos table is sharded with the tokens, so the
    device program is uniform SPMD with zero ragged logic.
  - Raw Bass (not Tile): this toolchain caps every instruction at ONE
    sync-wait command, so the pipeline is hand-built with per-ring-slot
    semaphores where each instruction carries at most one wait:
      sync   : DMA-in tile i    waits s_out[i%NB] (slot's previous store)
      vector : M1 waits s_in[i%NB]; S waits s_v (both muls done);
               A waits s_v (S done); standalone waits cover the sin|cos
               preload chunks and t_ac/t_bd reuse across tiles
      scalar : DMA-out tile i   waits s_cmp >= i+1
    Remaining hazards are covered transitively via semaphore clocks.
  - Per 128-token tile: one 2MB DMA in, 4 DVE ops using head-broadcast
    (stride-0) APs that write the rotated halves back in place, one 2MB
    DMA out. sin|cos lives SBUF-resident (preloaded once).
"""

import sys
import numpy as np

for _p in ("/opt/trn_rl_repo",):
    if _p not in sys.path:
        sys.path.insert(0, _p)

NC_COUNT = 8
P = 128  # SBUF partitions / tokens per tile
NB = 8  # x-tile ring depth
NPRE = 4  # sin|cos preload chunks (each with its own semaphore)

# Set True (module-level) to run with trace and stash the result object.
TRACE = False
LAST_RESULT = None


def _gather_rows(cache: np.ndarray, off: np.ndarray) -> np.ndarray:
    """Emulate JAX dynamic-gather semantics: negative indices wrapped once,
    then clipped into bounds."""
    n = cache.shape[0]
    idx = np.where(off < 0, off + n, off)
    idx = np.clip(idx, 0, n - 1)
    return cache[idx]


def _host_sincos(sin_cache, cos_cache, cu_seqlen, total):
    """Per-token sin|cos rows, replicating the reference exactly."""
    cu = np.asarray(cu_seqlen).astype(np.int64)
    tok = np.arange(total, dtype=np.int64)
    seg = np.searchsorted(cu, tok, side="right") - 1
    off = tok - cu[seg]  # numpy wraps seg=-1 like JAX does
    sin = _gather_rows(np.asarray(sin_cache, dtype=np.float32), off)
    cos = _gather_rows(np.asarray(cos_cache, dtype=np.float32), off)
    return np.concatenate([sin, cos], axis=1)  # [total, 2R]


def _build_program(ts, hd, h, r):
    """One-core program; SPMD-identical across the 8 cores.

    ts: tokens per core, hd: H*D, h: heads, r: rope dim (64).
    """
    from contextlib import ExitStack

    import concourse.bass as bass
    import concourse.mybir as mybir

    f32 = mybir.dt.float32
    half = r // 2  # 32
    d = hd // h  # head_dim
    ntiles = ts // P

    nc = bass.Bass()
    x_ext = nc.declare_dram_parameter("x", [ts, hd], f32, isOutput=False)
    sc_ext = nc.declare_dram_parameter("sc", [ts, 2 * r], f32, isOutput=False)
    out_ext = nc.declare_dram_parameter("out", [ts, hd], f32, isOutput=True)

    with ExitStack() as ctx:
        xbuf = ctx.enter_context(nc.sbuf_tensor([P, NB * hd], f32))
        sc_all = ctx.enter_context(nc.sbuf_tensor([P, ntiles * 2 * r], f32))
        t_ac = ctx.enter_context(nc.sbuf_tensor([P, h * 2 * half], f32))
        t_bd = ctx.enter_context(nc.sbuf_tensor([P, h * 2 * half], f32))
        # Per-ring-slot semaphores: at most one in-flight DMA per semaphore,
        # so "wait >= 16*k" exactly means "k-th DMA on this slot completed"
        # (increments from concurrent DMAs never interleave on one sem).
        s_pre = [ctx.enter_context(nc.semaphore(f"s_pre{k}")) for k in range(NPRE)]
        s_cmp = ctx.enter_context(nc.semaphore("s_cmp"))
        s_v = ctx.enter_context(nc.semaphore("s_v"))
        s_in = [ctx.enter_context(nc.semaphore(f"s_in{b}")) for b in range(NB)]
        s_out = [ctx.enter_context(nc.semaphore(f"s_out{b}")) for b in range(NB)]
        block = ctx.enter_context(nc.Block())

        sc_dst = sc_all[:].rearrange("p (n f) -> p n f", f=2 * r)
        sc_src = sc_ext[:].rearrange("(n p) f -> p n f", p=P)

        def xtile(i):
            b = i % NB
            return xbuf[:, b * hd : (b + 1) * hd]

        @block.sync
        def _(sync):
            for i in range(ntiles):
                dma = sync.dma_start(out=xtile(i), in_=x_ext[bass.ts(i, P), :])
                if i >= NB:
                    dma._wait_ge(s_out[i % NB], 16 * (i // NB))
                dma.then_inc(s_in[i % NB], 16)

        @block.vector
        def _(v):
            # DVE completion-order discipline: the engine pipelines, so RAW/
            # WAR between its own ops needs value-based waits on s_v (3 incs
            # per tile: M1, M2, S) / s_cmp (1 per tile: A). A sem's value
            # reaching a threshold joins ALL updaters' clocks, so each wait
            # below establishes happens-before to every prior op it covers.
            pre_chunk = ntiles // NPRE
            for i in range(ntiles):
                if i % pre_chunk == 0:
                    # this tile enters a new preload chunk of the sin|cos table
                    v.wait_ge(s_pre[i // pre_chunk], 16)
                if i >= 1:
                    # t_ac/t_bd reuse: wait until tile i-1's S and A are done
                    v.wait_ge(s_cmp, i)
                xt = xtile(i)
                x3 = xt.rearrange("p (h d) -> p h d", d=d)
                x1 = x3[:, :, 0:half]
                x2 = x3[:, :, half : 2 * half]
                sct = sc_dst[:, i, :]  # [P, 2r]: [sinL|sinH|cosL|cosH]

                # AC op: out[:,:,0,:] = x1*sinH ; out[:,:,1,:] = x1*cosL
                sc_ac = (
                    sct[:, half : 3 * half]
                    .rearrange("p (a d) -> p a d", d=half)
                    .unsqueeze(1)
                    .broadcast_to((P, h, 2, half))
                )
                # BD op: out[:,:,0,:] = x2*sinL ; out[:,:,1,:] = x2*cosH
                sc_bd = (
                    sct.rearrange("p (a d) -> p a d", d=half)[:, ::3, :]
                    .unsqueeze(1)
                    .broadcast_to((P, h, 2, half))
                )
                x1b = x1.unsqueeze(2).broadcast_to((P, h, 2, half))
                x2b = x2.unsqueeze(2).broadcast_to((P, h, 2, half))
                ac4 = t_ac[:].rearrange("p (h a d) -> p h a d", a=2, d=half)
                bd4 = t_bd[:].rearrange("p (h a d) -> p h a d", a=2, d=half)

                v.tensor_tensor(ac4, x1b, sc_ac, mybir.AluOpType.mult)._wait_ge(
                    s_in[i % NB], 16 * (i // NB + 1)
                ).then_inc(s_v, 1)
                v.tensor_tensor(bd4, x2b, sc_bd, mybir.AluOpType.mult).then_inc(
                    s_v, 1
                )
                # out[0:half] = x1*cosL - x2*sinL (into x1's slot)
                v.tensor_tensor(
                    x1, ac4[:, :, 1, :], bd4[:, :, 0, :], mybir.AluOpType.subtract
                )._wait_ge(s_v, 3 * i + 2).then_inc(s_v, 1)
                # out[half:r] = x2*cosH + x1*sinH (into x2's slot)
                v.tensor_tensor(
                    x2, bd4[:, :, 1, :], ac4[:, :, 0, :], mybir.AluOpType.add
                )._wait_ge(s_v, 3 * i + 3).then_inc(s_cmp, 1)

        @block.scalar
        def _(scalar):
            # Preload the per-core sin|cos table in NPRE chunks on the
            # (initially idle) store ring, token-major: partition p holds
            # tokens {p, P+p, 2P+p, ...}. Per-chunk semaphores keep each
            # sem single-writer (threshold crossings stay unambiguous).
            pre_chunk = ntiles // NPRE
            for k in range(NPRE):
                scalar.dma_start(
                    out=sc_dst[:, k * pre_chunk : (k + 1) * pre_chunk, :],
                    in_=sc_src[:, k * pre_chunk : (k + 1) * pre_chunk, :],
                ).then_inc(s_pre[k], 16)
            for i in range(ntiles):
                dma = scalar.dma_start(out=out_ext[bass.ts(i, P), :], in_=xtile(i))
                dma._wait_ge(s_cmp, i + 1)
                dma.then_inc(s_out[i % NB], 16)

    return nc


def kernel(input, sin_cache, cos_cache, cu_seqlen, max_seqlen=None, **_unused):
    global LAST_RESULT
    from concourse.bass_utils import run_bass_kernel_spmd

    x = np.asarray(input, dtype=np.float32)
    t, h, d = x.shape  # 32768, 32, 128
    r = np.asarray(sin_cache).shape[-1]  # 64
    hd = h * d
    assert t % (NC_COUNT * P) == 0
    ts = t // NC_COUNT

    sc = _host_sincos(sin_cache, cos_cache, cu_seqlen, t)  # [t, 2r] f32
    x2 = np.ascontiguousarray(x.reshape(t, hd))

    nc = _build_program(ts, hd, h, r)
    in_maps = [
        {
            "x": x2[i * ts : (i + 1) * ts],
            "sc": np.ascontiguousarray(sc[i * ts : (i + 1) * ts]),
        }
        for i in range(NC_COUNT)
    ]
    res = None
    last_exc = None
    for _attempt in range(3):
        try:
            res = run_bass_kernel_spmd(
                nc, in_maps, core_ids=list(range(NC_COUNT)), trace=TRACE
            )
            break
        except Exception as e:  # transient NRT/device wedges recover on retry
            last_exc = e
            import time as _time

            _time.sleep(5)
    if res is None:
        raise last_exc
    LAST_RESULT = res
    out = np.concatenate([res.results[i]["out"] for i in range(NC_COUNT)], axis=0)
    return out.reshape(t, h, d).astype(np.float32, copy=False)


# revision 19
# speedup vs baseline: 1.1311x; 1.1311x over previous
"""Ragged packed-sequence RoPE on 8 TRN2 NeuronCores.

Strategy:
  - Token-shard: core i gets tokens [i*T/8, (i+1)*T/8). The op is
    embarrassingly parallel over tokens; no collectives.
  - Raggedness is resolved on the host: per-token rotary rows are
    sin_cache[off]/cos_cache[off] where off restarts at 0 per sequence
    (exactly replicating the reference's searchsorted/gather semantics,
    including JAX's negative-wrap + clip out-of-bounds behavior). The
    gathered [T, 2*R] sin|cos table is sharded with the tokens, so the
    device program is uniform SPMD with zero ragged logic.
  - Raw Bass (not Tile): this toolchain caps every instruction at ONE
    sync-wait command, so the pipeline is hand-built with per-ring-slot
    semaphores where each instruction carries at most one wait:
      sync   : DMA-in tile i    waits s_out[i%NB] (slot's previous store)
      vector : M1 waits s_in[i%NB]; S waits s_v (both muls done);
               A waits s_v (S done); standalone waits cover the sin|cos
               preload chunks and t_ac/t_bd reuse across tiles
      scalar : DMA-out tile i   waits s_cmp >= i+1
    Remaining hazards are covered transitively via semaphore clocks.
  - Per 128-token tile: one 2MB DMA in, 4 DVE ops using head-broadcast
    (stride-0) APs that write the rotated halves back in place, one 2MB
    DMA out. sin|cos lives SBUF-resident (preloaded once).
"""

import sys
import numpy as np

for _p in ("/opt/trn_rl_repo",):
    if _p not in sys.path:
        sys.path.insert(0, _p)

NC_COUNT = 8
P = 128  # SBUF partitions / tokens per tile
NB = 8  # x-tile ring depth
NPRE = 4  # sin|cos preload chunks (each with its own semaphore)

# Set True (module-level) to run with trace and stash the result object.
TRACE = False
LAST_RESULT = None


def _gather_rows(cache: np.ndarray, off: np.ndarray) -> np.ndarray:
    """Emulate JAX dynamic-gather semantics: negative indices wrapped once,
    then clipped into bounds."""
    n = cache.shape[0]
    idx = np.where(off < 0, off + n, off)
    idx = np.clip(idx, 0, n - 1)
    return cache[idx]


def _host_sincos(sin_cache, cos_cache, cu_seqlen, total):
    """Per-token sin|cos rows, replicating the reference exactly."""
    cu = np.asarray(cu_seqlen).astype(np.int64)
    tok = np.arange(total, dtype=np.int64)
    seg = np.searchsorted(cu, tok, side="right") - 1
    off = tok - cu[seg]  # numpy wraps seg=-1 like JAX does
    sin = _gather_rows(np.asarray(sin_cache, dtype=np.float32), off)
    cos = _gather_rows(np.asarray(cos_cache, dtype=np.float32), off)
    return np.concatenate([sin, cos], axis=1)  # [total, 2R]


def _build_program(ts, hd, h, r):
    """One-core program; SPMD-identical across the 8 cores.

    ts: tokens per core, hd: H*D, h: heads, r: rope dim (64).
    """
    from contextlib import ExitStack

    import concourse.bass as bass
    import concourse.mybir as mybir

    f32 = mybir.dt.float32
    half = r // 2  # 32
    d = hd // h  # head_dim
    ntiles = ts // P

    nc = bass.Bass()
    x_ext = nc.declare_dram_parameter("x", [ts, hd], f32, isOutput=False)
    sc_ext = nc.declare_dram_parameter("sc", [ts, 2 * r], f32, isOutput=False)
    out_ext = nc.declare_dram_parameter("out", [ts, hd], f32, isOutput=True)

    with ExitStack() as ctx:
        xbuf = ctx.enter_context(nc.sbuf_tensor([P, NB * hd], f32))
        sc_all = ctx.enter_context(nc.sbuf_tensor([P, ntiles * 2 * r], f32))
        t_ac = ctx.enter_context(nc.sbuf_tensor([P, h * 2 * half], f32))
        t_bd = ctx.enter_context(nc.sbuf_tensor([P, h * 2 * half], f32))
        # Per-ring-slot semaphores: at most one in-flight DMA per semaphore,
        # so "wait >= 16*k" exactly means "k-th DMA on this slot completed"
        # (increments from concurrent DMAs never interleave on one sem).
        s_pre = [ctx.enter_context(nc.semaphore(f"s_pre{k}")) for k in range(NPRE)]
        s_cmp = ctx.enter_context(nc.semaphore("s_cmp"))
        s_v = ctx.enter_context(nc.semaphore("s_v"))
        s_in = [ctx.enter_context(nc.semaphore(f"s_in{b}")) for b in range(NB)]
        s_out = [ctx.enter_context(nc.semaphore(f"s_out{b}")) for b in range(NB)]
        block = ctx.enter_context(nc.Block())

        sc_dst = sc_all[:].rearrange("p (n f) -> p n f", f=2 * r)
        sc_src = sc_ext[:].rearrange("(n p) f -> p n f", p=P)

        def xtile(i):
            b = i % NB
            return xbuf[:, b * hd : (b + 1) * hd]

        @block.sync
        def _(sync):
            for i in range(ntiles):
                dma = sync.dma_start(out=xtile(i), in_=x_ext[bass.ts(i, P), :])
                if i >= NB:
                    dma._wait_ge(s_out[i % NB], 16 * (i // NB))
                dma.then_inc(s_in[i % NB], 16)

        @block.vector
        def _(v):
            # DVE completion-order discipline: the engine pipelines, so RAW/
            # WAR between its own ops needs value-based waits on s_v (3 incs
            # per tile: M1, M2, S) / s_cmp (1 per tile: A). A sem's value
            # reaching a threshold joins ALL updaters' clocks, so each wait
            # below establishes happens-before to every prior op it covers.
            pre_chunk = ntiles // NPRE
            for i in range(ntiles):
                if i % pre_chunk == 0:
                    # this tile enters a new preload chunk of the sin|cos table
                    v.wait_ge(s_pre[i // pre_chunk], 16)
                if i >= 1:
                    # t_ac/t_bd reuse: wait until tile i-1's S and A are done
                    v.wait_ge(s_cmp, i)
                xt = xtile(i)
                x3 = xt.rearrange("p (h d) -> p h d", d=d)
                x1 = x3[:, :, 0:half]
                x2 = x3[:, :, half : 2 * half]
                sct = sc_dst[:, i, :]  # [P, 2r]: [sinL|sinH|cosL|cosH]

                # AC op: out[:,:,0,:] = x1*sinH ; out[:,:,1,:] = x1*cosL
                sc_ac = (
                    sct[:, half : 3 * half]
                    .rearrange("p (a d) -> p a d", d=half)
                    .unsqueeze(1)
                    .broadcast_to((P, h, 2, half))
                )
                # BD op: out[:,:,0,:] = x2*sinL ; out[:,:,1,:] = x2*cosH
                sc_bd = (
                    sct.rearrange("p (a d) -> p a d", d=half)[:, ::3, :]
                    .unsqueeze(1)
                    .broadcast_to((P, h, 2, half))
                )
                x1b = x1.unsqueeze(2).broadcast_to((P, h, 2, half))
                x2b = x2.unsqueeze(2).broadcast_to((P, h, 2, half))
                ac4 = t_ac[:].rearrange("p (h a d) -> p h a d", a=2, d=half)
                bd4 = t_bd[:].rearrange("p (h a d) -> p h a d", a=2, d=half)

                v.tensor_tensor(ac4, x1b, sc_ac, mybir.AluOpType.mult)._wait_ge(
                    s_in[i % NB], 16 * (i // NB + 1)
                ).then_inc(s_v, 1)
                v.tensor_tensor(bd4, x2b, sc_bd, mybir.AluOpType.mult).then_inc(
                    s_v, 1
                )
                # out[0:half] = x1*cosL - x2*sinL (into x1's slot)
                v.tensor_tensor(
                    x1, ac4[:, :, 1, :], bd4[:, :, 0, :], mybir.AluOpType.subtract
                )._wait_ge(s_v, 3 * i + 2).then_inc(s_v, 1)
                # out[half:r] = x2*cosH + x1*sinH (into x2's slot)
                v.tensor_tensor(
                    x2, bd4[:, :, 1, :], ac4[:, :, 0, :], mybir.AluOpType.add
                )._wait_ge(s_v, 3 * i + 3).then_inc(s_cmp, 1)

        @block.scalar
        def _(scalar):
            # Preload the per-core sin|cos table in NPRE chunks on the
            # (initially idle) store ring, token-major: partition p holds
            # tokens {p, P+p, 2P+p, ...}. Per-chunk semaphores keep each
            # sem single-writer (threshold crossings stay unambiguous).
            pre_chunk = ntiles // NPRE
            for k in range(NPRE):
                scalar.dma_start(
                    out=sc_dst[:, k * pre_chunk : (k + 1) * pre_chunk, :],
                    in_=sc_src[:, k * pre_chunk : (k + 1) * pre_chunk, :],
                ).then_inc(s_pre[k], 16)
            for i in range(ntiles):
                dma = scalar.dma_start(out=out_ext[bass.ts(i, P), :], in_=xtile(i))
                dma._wait_ge(s_cmp, i + 1)
                dma.then_inc(s_out[i % NB], 16)

    return nc


def _run_on_device(x2, sc, ts, hd, h, r):
    """Build + run the SPMD program; returns the [t, hd] f32 output."""
    global LAST_RESULT
    from concourse.bass_utils import run_bass_kernel_spmd

    nc = _build_program(ts, hd, h, r)
    in_maps = [
        {
            "x": x2[i * ts : (i + 1) * ts],
            "sc": np.ascontiguousarray(sc[i * ts : (i + 1) * ts]),
        }
        for i in range(NC_COUNT)
    ]
    res = run_bass_kernel_spmd(
        nc, in_maps, core_ids=list(range(NC_COUNT)), trace=TRACE
    )
    LAST_RESULT = res
    return np.concatenate([res.results[i]["out"] for i in range(NC_COUNT)], axis=0)


def _run_with_recovery(x2, sc, ts, hd, h, r):
    """In-process attempts first; on persistent device errors (e.g. transient
    NRT_EXEC_UNIT_UNRECOVERABLE, which wedges the PJRT client for the life of
    the process), fall back to fresh subprocesses."""
    import time as _time

    last_exc = None
    for _attempt in range(2):
        try:
            return _run_on_device(x2, sc, ts, hd, h, r)
        except Exception as e:
            last_exc = e
            _time.sleep(5)

    import os
    import subprocess
    import tempfile

    for _attempt in range(2):
        tmpd = tempfile.mkdtemp(prefix="rope_sub_")
        in_path = os.path.join(tmpd, "in.npz")
        out_path = os.path.join(tmpd, "out.npy")
        np.savez(in_path, x2=x2, sc=sc, dims=np.array([ts, hd, h, r]))
        try:
            subprocess.run(
                [sys.executable, os.path.abspath(__file__), "--subprocess",
                 in_path, out_path],
                check=True,
                timeout=560,
                capture_output=True,
            )
            return np.load(out_path)
        except Exception as e:
            last_exc = e
            _time.sleep(5)
    raise last_exc


def kernel(input, sin_cache, cos_cache, cu_seqlen, max_seqlen=None, **_unused):
    x = np.asarray(input, dtype=np.float32)
    t, h, d = x.shape  # 32768, 32, 128
    r = np.asarray(sin_cache).shape[-1]  # 64
    hd = h * d
    assert t % (NC_COUNT * P) == 0
    ts = t // NC_COUNT

    sc = _host_sincos(sin_cache, cos_cache, cu_seqlen, t)  # [t, 2r] f32
    x2 = np.ascontiguousarray(x.reshape(t, hd))

    out = _run_with_recovery(x2, sc, ts, hd, h, r)
    return out.reshape(t, h, d).astype(np.float32, copy=False)


if __name__ == "__main__" and len(sys.argv) >= 4 and sys.argv[1] == "--subprocess":
    _d = np.load(sys.argv[2])
    _ts, _hd, _h, _r = (int(v) for v in _d["dims"])
    _out = _run_on_device(_d["x2"], _d["sc"], _ts, _hd, _h, _r)
    np.save(sys.argv[3], _out)
